# revision 15
# baseline (speedup 1.0000x reference)
"""Trainium2 Bass kernel for nn_CombineUV (shortlist-scored retrieval).

Math: out[b,s] = dot(input[b], sig(alpha)*weight[i] + sig(beta)*labels[i]) + bias[i]
with i = shortlist[b,s].  Folded into the input side:
out[b,s] = dot(xa[b], weight[i]) + dot(xb[b], labels[i]) + bias[i],
xa = input*sig(alpha), xb = input*sig(beta).

Device strategy (8 cores, L-sharded):
 - Stream: one fp8e3 (e3m4) column per distinct table row hit on this core:
   8 chunks of 128 contraction dims; chunks 0-3 hold weight*WS, 4-7 hold
   labels*LS (scales put values in e3m4's normal range); the xc side carries
   xa/WS, xb/LS so PSUM dots come out in original units.  fp8 halves the
   dominant HBM stream vs bf16 at ~1.2e-2 worst-case relative error.
 - Batches split into 16 groups of 32.  A PASS loads 4 group slices as four
   col-tiled stationaries (tile_position=(0,32k), lhsT [128,32] bf16) and
   streams the tile's fp8 columns once: 32 accumulating matmuls running
   4-way concurrent in the PE -> per-pass cost ~8*w cycles, same as a
   128-batch window, but the 4 groups are chosen per tile.  Columns are
   clustered so most tiles need a single pass (vs 1.82 fixed-quarter
   windows in the old design).
 - Packing: stage A = AG(2,4) affine-plane lines (every pair of groups has
   a unique 4-line home) + greedy exact 4-sets, 1 pass each; stage B =
   greedy 8-sets, 2 passes, the second pass streaming only the prefix of
   columns that need it; stage C = 16-group catch-all tiles with interval
   passes.
 - Output: each pass's PSUM [128, w] (all 128 batch x w column dots) is
   copied to fp16 SBUF (alternating ScalarE/VectorE) and DMA'd out whole.
   The host gathers the ~2.05 hit pairs per column straight from the pass
   output (no masks / reduce matmuls / spill columns needed) and adds
   bias[shortlist].
"""

import sys

sys.path.insert(0, "/opt/trn_rl_repo")

import numpy as np
import ml_dtypes
from itertools import combinations

BF16 = ml_dtypes.bfloat16
F8E3 = ml_dtypes.float8_e3m4
F16 = np.float16

L, D, B, S = 131072, 512, 512, 512
NCORES = 8
LSH = L // NCORES
NCHUNK = 8
G = 32                     # batch group width
NG = B // G                # 16 groups
TILE = 512
WGRAN = 16
WS = 256.0                 # weight-half fp8 scale
LS = 32.0                  # labels-half fp8 scale
TH_A = 230                 # stage-A greedy: min avg coverage per core
TH_B = 48                  # stage-B greedy: min avg coverage per core

_PROG_CACHE = {}


def _emit_columns(lidx, bvec, pos):
    """One column per distinct table row on this core.

    Returns list of (groupmask, row, served) with
    served = {g: [(m, flatpos), ...]} (m = batch % 32), no pair limit.
    """
    order = np.lexsort((bvec, lidx))
    li, bv, ps = lidx[order], bvec[order], pos[order]
    cols = []
    n = len(li)
    i = 0
    while i < n:
        j = i
        while j < n and li[j] == li[i]:
            j += 1
        served = {}
        mask = 0
        for k in range(i, j):
            g = int(bv[k]) // G
            served.setdefault(g, []).append((int(bv[k]) % G, int(ps[k])))
            mask |= 1 << g
        cols.append((mask, int(li[i]), served))
        i = j
    return cols


def _pack_structure(cols_by_core):
    """Cluster columns into tiles of pass schedules (shared across cores).

    Returns (tiles, percore, total_w8, nwin): tiles have
      passes/pinfo: [(slots 4-tuple, wp pass width, wi out index)], w, st_off
    and percore[c][t] = column list for tile t on core c.
    """
    buckets = []
    for cols in cols_by_core:
        d = {}
        for col in cols:
            d.setdefault(col[0], []).append(col)
        buckets.append(d)
    allS = np.array(sorted(set(m for d in buckets for m in d)), dtype=np.int64)
    NS = len(allS)
    sidx = {int(s): i for i, s in enumerate(allS)}
    ssize = np.array([bin(int(s)).count("1") for s in allS])
    cnt = np.zeros((NCORES, NS), dtype=np.int64)
    for c, d in enumerate(buckets):
        for m, lst in d.items():
            cnt[c, sidx[m]] = len(lst)

    cand4 = np.array(
        [sum(1 << g for g in cb) for cb in combinations(range(NG), 4)],
        dtype=np.int64,
    )
    sub4 = (allS[:, None] & ~cand4[None, :]) == 0

    # AG(2,4) affine plane: every pair of groups lies on exactly one line
    M4 = [[0, 0, 0, 0], [0, 1, 2, 3], [0, 2, 3, 1], [0, 3, 1, 2]]
    lines = []
    for m in range(4):
        for bb in range(4):
            lines.append(sum(1 << (x * 4 + (M4[m][x] ^ bb)) for x in range(4)))
    for cc in range(4):
        lines.append(sum(1 << (cc * 4 + y) for y in range(4)))
    cand4L = np.array(lines, dtype=np.int64)
    sub4L = (allS[:, None] & ~cand4L[None, :]) == 0

    tiles = []
    percore = [[] for _ in range(NCORES)]

    def take(c, eligible, room):
        out = []
        for si in eligible:
            if room == 0:
                break
            m = int(allS[si])
            lst = buckets[c].get(m)
            n = cnt[c, si]
            if not n:
                continue
            k = min(room, n)
            out.extend(lst[-k:])
            del lst[-k:]
            cnt[c, si] -= k
            room -= k
        return out

    def stageA(c4, s4, thresh):
        s4f = s4.astype(np.float32)
        while True:
            cov = np.minimum(cnt.astype(np.float32) @ s4f, TILE).sum(axis=0)
            best = int(cov.argmax())
            if cov[best] < NCORES * thresh:
                break
            bm = int(c4[best])
            elig = np.nonzero(s4[:, best])[0]
            elig = elig[np.argsort(-ssize[elig])]
            maxw = 0
            for c in range(NCORES):
                got = take(c, elig, TILE)
                percore[c].append(got)
                maxw = max(maxw, len(got))
            if maxw == 0:
                for c in range(NCORES):
                    percore[c].pop()
                break
            tiles.append({"kind": "A", "sets": [bm]})

    stageA(cand4L, sub4L, 56)
    stageA(cand4, sub4, TH_A)

    # stage B: 8-sets, 2 passes, second pass prefix-width
    cand8 = np.array(
        [sum(1 << g for g in cb) for cb in combinations(range(NG), 8)],
        dtype=np.int64,
    )
    sub8 = (allS[:, None] & ~cand8[None, :]) == 0
    sub8f = sub8.astype(np.float32)
    while True:
        cov = np.minimum(cnt.astype(np.float32) @ sub8f, TILE).sum(axis=0)
        best = int(cov.argmax())
        if cov[best] < NCORES * TH_B:
            break
        bm = int(cand8[best])
        elig = np.nonzero(sub8[:, best])[0]
        elig = elig[np.argsort(-ssize[elig])]
        maxw = 0
        for c in range(NCORES):
            got = take(c, elig, TILE)
            percore[c].append(got)
            maxw = max(maxw, len(got))
        if maxw == 0:
            for c in range(NCORES):
                percore[c].pop()
            break
        tiles.append({"kind": "B", "sets": [bm]})

    # stage C: catch-all
    while cnt.sum() > 0:
        elig = np.argsort(-ssize)
        maxw = 0
        for c in range(NCORES):
            got = take(c, elig, TILE)
            percore[c].append(got)
            maxw = max(maxw, len(got))
        if maxw == 0:
            break
        tiles.append({"kind": "C", "sets": [(1 << NG) - 1]})

    # finalize pass schedules
    for t, tl in enumerate(tiles):
        if tl["kind"] == "A":
            bm = tl["sets"][0]
            slots = tuple(g for g in range(NG) if bm >> g & 1)
            w = max(len(percore[c][t]) for c in range(NCORES))
            w = max(WGRAN, -(-w // WGRAN) * WGRAN)
            tl["passes"] = [(slots, w)]
            tl["w"] = w
        elif tl["kind"] == "B":
            bm = tl["sets"][0]
            groups = [g for g in range(NG) if bm >> g & 1]
            colm = [
                np.array([col[0] for col in percore[c][t]], dtype=np.int64)
                for c in range(NCORES)
            ]
            best_split, best_w2 = None, None
            for cb in combinations(range(8), 4):
                g1 = sum(1 << groups[i] for i in cb)
                g2 = bm & ~g1
                w2 = 0
                for c in range(NCORES):
                    if len(colm[c]):
                        w2 = max(w2, int(np.count_nonzero(colm[c] & g2)))
                if best_w2 is None or w2 < best_w2:
                    best_w2, best_split = w2, g1
            g1 = best_split
            g2 = bm & ~g1
            for c in range(NCORES):
                percore[c][t].sort(key=lambda col: -(1 if col[0] & g2 else 0))
            w = max(len(percore[c][t]) for c in range(NCORES))
            w = max(WGRAN, -(-w // WGRAN) * WGRAN)
            w2 = max(WGRAN, -(-best_w2 // WGRAN) * WGRAN) if best_w2 else 0
            slots1 = tuple(g for g in range(NG) if g1 >> g & 1)
            slots2 = tuple(g for g in range(NG) if g2 >> g & 1)
            tl["passes"] = [(slots1, w)] + ([(slots2, min(w2, w))] if w2 else [])
            tl["w"] = w
        else:
            def homemask(col):
                hm = 0
                for g in col[2]:
                    hm |= 1 << (g // 4)
                return hm
            for c in range(NCORES):
                percore[c][t].sort(key=lambda col: -bin(homemask(col)).count("1"))
            w = max(len(percore[c][t]) for c in range(NCORES))
            w = max(WGRAN, -(-w // WGRAN) * WGRAN)
            passes = []
            for h in range(4):
                wp = 0
                for c in range(NCORES):
                    hit = [i for i, col in enumerate(percore[c][t])
                           if any(g // 4 == h for g in col[2])]
                    if hit:
                        wp = max(wp, hit[-1] + 1)
                if wp:
                    wp = max(WGRAN, -(-wp // WGRAN) * WGRAN)
                    passes.append((tuple(range(4 * h, 4 * h + 4)), min(wp, w)))
            tl["passes"] = passes
            tl["w"] = w

    keep = [t for t, tl in enumerate(tiles) if tl["w"] > 0 and tl["passes"]]
    tiles = [tiles[t] for t in keep]
    for c in range(NCORES):
        percore[c] = [percore[c][t] for t in keep]

    # order: B first (2 passes per stream load fill the pipeline), C mid,
    # then A by width desc so the narrowest tiles drain the tail
    order = sorted(
        range(len(tiles)),
        key=lambda t: (
            {"B": 0, "A": 1, "C": 2}[tiles[t]["kind"]],
            -tiles[t]["w"],
        ),
    )
    tiles = [tiles[t] for t in order]
    for c in range(NCORES):
        percore[c] = [percore[c][t] for t in order]

    st_off = 0
    wi = 0
    for tl in tiles:
        tl["st_off"] = st_off
        st_off += NCHUNK * tl["w"]
        pinfo = []
        for slots4, wp in tl["passes"]:
            pinfo.append({"slots": slots4, "wp": wp, "wi": wi})
            wi += 1
        tl["pinfo"] = pinfo
    return tiles, percore, st_off, wi


def _build_maps(tiles, percore, total_w8, nwin, TC):
    """Per-core stream arrays + host gather index lists."""
    in_maps, gathers = [], []
    for c in range(NCORES):
        st = np.zeros((128, total_w8), dtype=F8E3)
        dev_idx, fpos_idx = [], []
        for t, tl in enumerate(tiles):
            cols = percore[c][t]
            w = tl["w"]
            if cols:
                rows = np.array([r for _, r, _ in cols], np.int64)
                arr = TC[c * LSH : (c + 1) * LSH][rows]
                arr = arr.reshape(len(rows), NCHUNK, 128)
                st[:, tl["st_off"] : tl["st_off"] + NCHUNK * w].reshape(
                    128, NCHUNK, w
                )[:, :, : len(rows)] = arr.transpose(2, 1, 0)
            # route each column-group incidence to the first pass covering it
            for j, (_, _, served) in enumerate(cols):
                for g, pairs in served.items():
                    for pi in tl["pinfo"]:
                        if g in pi["slots"] and j < pi["wp"]:
                            k = pi["slots"].index(g)
                            base = pi["wi"] * (128 * TILE) + (32 * k) * TILE + j
                            for m, fp in pairs:
                                dev_idx.append(base + m * TILE)
                                fpos_idx.append(fp)
                            break
                    else:
                        raise AssertionError("uncovered column-group incidence")
        in_maps.append({"st": st})
        gathers.append(
            (np.array(dev_idx, np.int64), np.array(fpos_idx, np.int64))
        )
    return in_maps, gathers


def _build_program(sig, tiles, total_w8, nwin):
    import concourse.bacc as bacc
    import concourse.mybir as mybir
    from concourse.tile import TileContext

    f32, bf = mybir.dt.float32, mybir.dt.bfloat16
    f8, f16 = mybir.dt.float8e3, mybir.dt.float16

    nc = bacc.Bacc(None, target_bir_lowering=False)
    st_d = nc.dram_tensor("st", [128, total_w8], f8, kind="ExternalInput")
    xc_d = nc.dram_tensor("xc", [128, NCHUNK * B], bf, kind="ExternalInput")
    out_d = nc.dram_tensor("out", [nwin, 128, TILE], f16, kind="ExternalOutput")

    with TileContext(nc) as tc:
        with (
            tc.tile_pool(name="res", bufs=1) as res_pool,
            tc.tile_pool(name="g", bufs=8) as gpool,
            tc.tile_pool(name="o", bufs=6) as opool,
            tc.tile_pool(name="ps", bufs=7, space="PSUM") as pspool,
        ):
            xcq = [
                res_pool.tile(
                    [128, NCHUNK * 4 * G], bf, tag=f"xcq{k}", name=f"xcq{k}"
                )
                for k in range(4)
            ]
            for k in range(4):
                # scalar queue: don't delay the first stream tiles on sync
                nc.scalar.dma_start(
                    out=xcq[k][:],
                    in_=xc_d[:, k * 1024 : (k + 1) * 1024],
                )

            # PE warmup during the DMA ramp: ~3.4us of dummy matmuls flips
            # the HAM clock gate to 8/8 before the first real pass lands.
            warm = res_pool.tile([128, TILE], bf, tag="warm", name="warm")
            nc.gpsimd.memset(warm[:], 0)
            wps = pspool.tile([128, TILE], f32, tag="ps")
            for _ in range(8):
                nc.tensor.matmul(
                    out=wps[:, :TILE],
                    lhsT=warm[:, :128],
                    rhs=warm[:, :TILE],
                    start=True,
                    stop=True,
                )

            pending = None
            npass_done = 0

            def evac(ps, wp, wi, ncopy):
                ot = opool.tile([128, TILE], f16, tag="o")
                half = max(WGRAN, (wp // 2 // WGRAN) * WGRAN)
                half = min(half, wp)
                # split the PSUM evacuation across both ACT and DVE
                if half and half < wp:
                    nc.scalar.copy(ot[:, :half], ps[:, :half])
                    nc.vector.tensor_scalar_add(
                        ot[:, half:wp], ps[:, half:wp], 0.0
                    )
                elif ncopy % 2:
                    nc.scalar.copy(ot[:, :wp], ps[:, :wp])
                else:
                    nc.vector.tensor_scalar_add(ot[:, :wp], ps[:, :wp], 0.0)
                eng = (nc.scalar, nc.gpsimd, nc.sync)[ncopy % 3]
                eng.dma_start(out=out_d[wi][:, :wp], in_=ot[:, :wp])

            for t, tl in enumerate(tiles):
                w = tl["w"]
                g = gpool.tile([128, NCHUNK * TILE], f8, tag="g")
                nsplit = 4 if t < 2 else (2 if t < 4 else 1)
                step = (NCHUNK // nsplit) * w
                # alternate issue engine so dma_start dispatch (~650ns each)
                # doesn't serialize the prefetch ramp on one queue
                leng = nc.sync if t % 2 == 0 else nc.scalar
                for k in range(nsplit):
                    leng.dma_start(
                        out=g[:, k * step : (k + 1) * step],
                        in_=st_d[
                            :,
                            tl["st_off"] + k * step : tl["st_off"] + (k + 1) * step,
                        ],
                    )
                for pi in tl["pinfo"]:
                    slots4, wp = pi["slots"], pi["wp"]
                    ps = pspool.tile([128, TILE], f32, tag="ps")
                    for c in range(NCHUNK):
                        for k in range(len(slots4)):
                            gg = slots4[k]
                            nc.tensor.matmul(
                                out=ps[32 * k : 32 * k + 32, :wp],
                                lhsT=xcq[gg // 4][
                                    :,
                                    (gg % 4) * (NCHUNK * G) + c * G :
                                    (gg % 4) * (NCHUNK * G) + (c + 1) * G,
                                ],
                                rhs=g[:, c * w : c * w + wp],
                                start=(c == 0),
                                stop=(c == NCHUNK - 1),
                                tile_position=(0, 32 * k),
                            )
                    if pending is not None:
                        evac(*pending, npass_done)
                        npass_done += 1
                    pending = (ps, wp, pi["wi"])
            if pending is not None:
                evac(*pending, npass_done)

    nc.compile()
    return nc


def _prep_inputs(input, labels, weight, alpha, beta, shortlist):
    input = np.asarray(input, dtype=np.float32)
    alpha = np.asarray(alpha, dtype=np.float32).reshape(1, D)
    beta = np.asarray(beta, dtype=np.float32).reshape(1, D)
    xa = input * (1.0 / (1.0 + np.exp(-alpha))) / WS
    xb = input * (1.0 / (1.0 + np.exp(-beta))) / LS

    XC = np.empty((128, NG, NCHUNK, G), dtype=BF16)
    XC[:, :, :4, :] = xa.reshape(NG, G, 4, 128).transpose(3, 0, 2, 1)
    XC[:, :, 4:, :] = xb.reshape(NG, G, 4, 128).transpose(3, 0, 2, 1)
    xc_flat = np.ascontiguousarray(XC.reshape(128, NCHUNK * B))

    TC = np.concatenate(
        [
            np.clip(np.asarray(weight, np.float32) * WS, -15.5, 15.5),
            np.clip(np.asarray(labels, np.float32) * LS, -15.5, 15.5),
        ],
        axis=1,
    ).astype(F8E3)  # [L, 1024]

    sl = np.asarray(shortlist).reshape(-1).astype(np.int64)
    core = sl // LSH
    lidx = sl % LSH
    bvec = np.repeat(np.arange(B, dtype=np.int64), S)
    allpos = np.arange(B * S, dtype=np.int64)

    cols_by_core = []
    for c in range(NCORES):
        m = core == c
        cols_by_core.append(_emit_columns(lidx[m], bvec[m], allpos[m]))

    tiles, percore, total_w8, nwin = _pack_structure(cols_by_core)
    in_maps, gathers = _build_maps(tiles, percore, total_w8, nwin, TC)
    for c in range(NCORES):
        in_maps[c]["xc"] = xc_flat

    sig = tuple(
        (tuple(pi["slots"]), pi["wp"]) for tl in tiles for pi in tl["pinfo"]
    ) + (total_w8,)
    return sig, tiles, total_w8, nwin, in_maps, gathers


def kernel(input, labels, weight, alpha, beta, bias, shortlist, _trace=False):
    from concourse.bass_utils import run_bass_kernel_spmd

    (sig, tiles, total_w8, nwin, in_maps, gathers) = _prep_inputs(
        input, labels, weight, alpha, beta, shortlist
    )

    if sig not in _PROG_CACHE:
        _PROG_CACHE[sig] = _build_program(sig, tiles, total_w8, nwin)
    nc = _PROG_CACHE[sig]

    res = run_bass_kernel_spmd(nc, in_maps, list(range(NCORES)), trace=_trace)

    out_flat = np.zeros(B * S, dtype=np.float32)
    for c in range(NCORES):
        vals = np.asarray(res.results[c]["out"]).reshape(-1)
        dev_idx, fpos_idx = gathers[c]
        out_flat[fpos_idx] = vals[dev_idx].astype(np.float32)

    bias = np.asarray(bias, dtype=np.float32)
    sl = np.asarray(shortlist).reshape(-1).astype(np.int64)
    out_flat += bias[sl]
    out = out_flat.reshape(B, S)

    if _trace:
        return out, res
    return out


if __name__ == "__main__":
    import jax
    import time

    key = jax.random.key(0)
    k0, k1, k2, k3 = jax.random.split(key, 4)
    shortlist = np.asarray(jax.random.randint(k3, (B, S), 0, L, dtype=np.int64))
    sl = shortlist.reshape(-1)
    core = sl // LSH
    lidx = sl % LSH
    bvec = np.repeat(np.arange(B, dtype=np.int64), S)
    allpos = np.arange(B * S, dtype=np.int64)

    t0 = time.time()
    cols_by_core = [
        _emit_columns(lidx[core == c], bvec[core == c], allpos[core == c])
        for c in range(NCORES)
    ]
    t1 = time.time()
    tiles, percore, total_w8, nwin = _pack_structure(cols_by_core)
    t2 = time.time()
    print(f"emit {t1-t0:.1f}s pack {t2-t1:.1f}s")
    print(f"tiles={len(tiles)} nwin={nwin}")
    print(f"stream cols={total_w8 // NCHUNK} bytes/core={total_w8*128/1e6:.1f}MB")
    pe = sum(
        8 * pi["wp"] / 2.4 + 350 for tl in tiles for pi in tl["pinfo"]
    )
    outb = sum(128 * TILE * 2 for tl in tiles for pi in tl["pinfo"])
    print(f"predicted PE: {pe/1e3:.1f}us  out {outb/1e6:.2f}MB")
    kinds = {}
    for tl in tiles:
        kinds[tl["kind"]] = kinds.get(tl["kind"], 0) + 1
    print("tile kinds:", kinds)
